# revision 44
# baseline (speedup 1.0000x reference)
"""Sliding-window (W=128) multi-head attention block for Trainium2, 8 cores.

Reference computation (B=2, T=2048, E=1024, H=16, D=64, W=128):
    qkv = x @ w_qkv.T ; split q,k,v ; heads ; att = softmax(mask(q k^T / 8)) v
    out = att_concat @ w_out.T

Sharding: data-parallel over B (2) x tensor-parallel over head groups (4),
so each of the 8 cores handles (one batch, 4 heads).  The output projection
is computed per-core against the 256 w_out columns belonging to its heads,
giving a partial [T, E] output (bf16); the host sums the 4 partials per
batch in f32.

Attention is computed in "transposed score" form to avoid PE transposes:
    S^T[k, q] = k^T.T @ q^T   (keys on partitions, d=64 contraction)
    E^T = exp(S^T)            (ACT, psum->sbuf bf16)
    mask -> 0 via affine_select on GpSimd (band structure is affine)
    [O'^T ; l] = [V | 1].T @ E^T  (l = softmax denominator, folded into the
                                   O matmul via ones-columns in the weights)
    attT = O'^T * (1/l)       (DVE reciprocal + multiply, psum in)
The two heads of a q^T/k^T pair chunk live at partitions 0:64 / 64:128 and
their K=64 S^T matmuls are row-tiled (tile_position auto-derived from the
base partition), so the pair runs concurrently on the PE array.

The 1/sqrt(D) scale is folded into the q weights on the host.
"""

import numpy as np
import ml_dtypes

import concourse.bass as bass
import concourse.bacc as bacc
import concourse.mybir as mybir
import concourse.tile as tile
from concourse.bass_utils import run_bass_kernel_spmd

B, T, E, H, W = 2, 2048, 1024, 16, 128
D = E // H            # 64
HPC = 4               # heads per core
N_CORES = 8
SCALE = 1.0 / float(np.sqrt(D))

BF16 = mybir.dt.bfloat16
F32 = mybir.dt.float32

KO = E // 128         # 8 contraction chunks
NQT = T // 128        # 16 query tiles
NT512 = T // 512      # 4 tiles for the projections

# Row-tiled K=64 S^T matmuls (pair-concurrent).  False falls back to
# zero-padded K=128 per-head kT (no partition-offset PE operands).
ROWPACK = True


def build_bass():
    nc = bacc.Bacc()
    # inputs prepacked on the host into SBUF layout ([p, ko, minor] row-
    # major) so every input DMA reads >=4KB contiguous runs per partition
    # (256-col rearranged slices of the natural layouts measured ~118GB/s
    # vs ~320GB/s for contiguous) and can be ordered by first use
    wqkk2 = nc.declare_dram_parameter("wqkk2", [128, KO * 128], BF16, isOutput=False)
    wqkk3 = nc.declare_dram_parameter("wqkk3", [128, KO * 128], BF16, isOutput=False)
    wqkq = nc.declare_dram_parameter("wqkq", [128, KO * 256], BF16, isOutput=False)
    wvp = nc.declare_dram_parameter("wvp", [128, KO * 256], BF16, isOutput=False)
    x0a = nc.declare_dram_parameter("x0a", [128, KO * 256], BF16, isOutput=False)
    x0b = nc.declare_dram_parameter("x0b", [128, KO * 256], BF16, isOutput=False)
    x1 = nc.declare_dram_parameter("x1", [128, KO * 512], BF16, isOutput=False)
    x2 = nc.declare_dram_parameter("x2", [128, KO * 512], BF16, isOutput=False)
    x3 = nc.declare_dram_parameter("x3", [128, KO * 512], BF16, isOutput=False)
    wout = nc.declare_dram_parameter("wout", [HPC * D, E], BF16, isOutput=False)
    outp = nc.declare_dram_parameter("outp", [T, E], BF16, isOutput=True)

    with tile.TileContext(nc) as tc:
        with (
            tc.tile_pool(name="persist", bufs=1) as persist,
            tc.tile_pool(name="work", bufs=3) as work,
            tc.tile_pool(name="rlp", bufs=2) as rlp,
            tc.tile_pool(name="outw", bufs=3) as outw,
            tc.tile_pool(name="ps_mm", bufs=2, space="PSUM") as ps_mm,
            tc.tile_pool(name="ps_s", bufs=2, space="PSUM") as ps_s,
            tc.tile_pool(name="ps_o", bufs=2, space="PSUM") as ps_o,
        ):
            # ---- persistent tiles ----
            wqk_sb = persist.tile([128, KO, 2 * HPC * D], BF16)
            wv_sb = persist.tile([128, KO, HPC * D], BF16)
            wout_sb = persist.tile([128, 2, E], BF16)
            xT_sb = persist.tile([128, KO, T], BF16)
            qkT_sb = persist.tile([128, 2, T], BF16)   # q^T pairs (scaled)
            if ROWPACK:
                kT_sb = persist.tile([128, 2, T], BF16)  # k^T pairs
            else:
                kT_sb = persist.tile([128, HPC, T], BF16)  # zero-padded k^T
                nc.vector.memset(kT_sb, 0.0)
            # V augmented with ones columns: [:, kt, h, 0:64]=V_h, 64:128=1
            vA_sb = persist.tile([128, NQT, HPC, 128], BF16)
            nc.gpsimd.memset(vA_sb[:, :, :, D:128], 1.0)
            attT_sb = persist.tile([128, 2, T], BF16)  # O^T, chunk j: heads 2j,2j+1

            # ---- PE warmup: dependency-free matmuls release the HAM clock
            # gate (K=4/8 -> 8/8 needs ~3.4us of sustained PE activity)
            # while the first input DMAs are still in flight ----
            wu_sb = persist.tile([128, 512], BF16)
            nc.vector.memset(wu_sb, 0.0)
            for wi in range(9):
                wu_ps = ps_o.tile([128, HPC, 128], F32, tag="O")
                nc.tensor.matmul(
                    wu_ps.rearrange("p a b -> p (a b)"), lhsT=wu_sb[:, 0:128],
                    rhs=wu_sb, start=True, stop=True)

            # ---- input DMAs, ordered by first use so the PE can stream
            # just behind the HBM reads (~320 GB/s): the ti=0 k-side
            # chunks (mi 2,3) need only wqk cols 256:512 + x tokens; the
            # token halves let the first matmuls start after ~1MB ----
            def ap3(t, m):
                return t[:, :].rearrange("p (ko m) -> p ko m", m=m)
            nc.sync.dma_start(out=wqk_sb[:, :, 256:384], in_=ap3(wqkk2, 128))
            nc.sync.dma_start(out=xT_sb[:, :, 0:256], in_=ap3(x0a, 256))
            nc.sync.dma_start(out=wqk_sb[:, :, 384:512], in_=ap3(wqkk3, 128))
            nc.sync.dma_start(out=xT_sb[:, :, 256:512], in_=ap3(x0b, 256))
            nc.sync.dma_start(out=wqk_sb[:, :, 0:256], in_=ap3(wqkq, 256))
            nc.sync.dma_start(out=wv_sb, in_=ap3(wvp, 256))
            nc.sync.dma_start(out=xT_sb[:, :, 512:1024], in_=ap3(x1, 512))
            nc.sync.dma_start(
                out=wout_sb, in_=wout[:, :].rearrange("(c p) m -> p c m", p=128))
            nc.sync.dma_start(out=xT_sb[:, :, 1024:1536], in_=ap3(x2, 512))
            nc.sync.dma_start(out=xT_sb[:, :, 1536:2048], in_=ap3(x3, 512))

            def qk_finish(ti, mi, ps):
                tsl = slice(ti * 512, (ti + 1) * 512)
                if mi < 2:
                    nc.vector.tensor_copy(out=qkT_sb[:, mi, tsl], in_=ps)
                elif ROWPACK:
                    nc.scalar.copy(out=kT_sb[:, mi - 2, tsl], in_=ps)
                else:
                    hp = (mi - 2) * 2
                    nc.scalar.copy(out=kT_sb[0:64, hp, tsl], in_=ps[0:64])
                    nc.scalar.copy(
                        out=kT_sb[64:128, hp + 1, tsl], in_=ps[64:128])

            def qk_chunk(ti, mi):
                ps = ps_mm.tile([128, 512], F32, tag="mm")
                for ko in range(KO):
                    nc.tensor.matmul(
                        ps,
                        lhsT=wqk_sb[:, ko, mi * 128:(mi + 1) * 128],
                        rhs=xT_sb[:, ko, ti * 512:(ti + 1) * 512],
                        start=(ko == 0), stop=(ko == KO - 1),
                    )
                qk_finish(ti, mi, ps)

            def qk_khalves0():
                # ti=0 k-chunks (mi 2,3) in N=256 token-halves, interleaved
                # so the PE streams just behind the input DMA: mi2 needs
                # only wqk cols 256:512 + x tokens 0:256 for its first MMs
                pss = {mi: ps_mm.tile([128, 512], F32, tag="mm",
                                      name=f"ps_kh{mi}")
                       for mi in (2, 3)}
                for lo, hi in ((0, 256), (256, 512)):
                    for mi in (2, 3):
                        for ko in range(KO):
                            nc.tensor.matmul(
                                pss[mi][:, lo:hi],
                                lhsT=wqk_sb[:, ko, mi * 128:(mi + 1) * 128],
                                rhs=xT_sb[:, ko, lo:hi],
                                start=(ko == 0), stop=(ko == KO - 1),
                            )
                for mi in (2, 3):
                    qk_finish(0, mi, pss[mi])

            def v_chunk(ti, j):
                t0 = ti * 512 + j * 128
                ps = ps_mm.tile([128, 512], F32, tag="mm")
                for ko in range(KO):
                    nc.tensor.matmul(
                        ps[:, 0:HPC * D],
                        lhsT=xT_sb[:, ko, t0:t0 + 128],
                        rhs=wv_sb[:, ko, :],
                        start=(ko == 0), stop=(ko == KO - 1),
                    )
                nc.vector.tensor_copy(
                    out=vA_sb[:, ti * 4 + j, :, 0:D],
                    in_=ps[:, 0:HPC * D])

            def emit_S(qi, split=False):
                """S^T matmuls + exp + band mask; returns E^T sbuf tile
                laid out [128 k, par, ci, mi, q] (head h = 2*mi + par).
                split: per-par exp/mask so par-0's O matmuls can start
                before par-1's softmax chain finishes (tail latency)."""
                qsl = slice(qi * 128, (qi + 1) * 128)
                cis = [1] if qi == 0 else [0, 1]
                psS = [ps_s.tile([128, 2, 2, 128], F32, tag=f"S{par}",
                                 name=f"psS{par}")
                       for par in range(2)]
                for ci in cis:
                    kt = qi - 1 + ci
                    ksl = slice(kt * 128, (kt + 1) * 128)
                    for mi in range(2):
                        for par in range(2):
                            prows = slice(par * 64, par * 64 + 64)
                            nc.tensor.matmul(
                                psS[par][:, ci, mi, :],
                                lhsT=kT_sb[prows, mi, ksl],
                                rhs=qkT_sb[prows, mi, qsl],
                                start=True, stop=True,
                            )
                esb = work.tile([128, 2, 2, 2, 128], BF16, tag="E")
                for par in range(2):
                    if qi == 0:
                        nc.scalar.activation(
                            out=esb[:, par, 1, :, :], in_=psS[par][:, 1, :, :],
                            func=mybir.ActivationFunctionType.Exp)
                    else:
                        nc.scalar.activation(
                            out=esb[:, par, :, :, :], in_=psS[par],
                            func=mybir.ActivationFunctionType.Exp)
                    if split:
                        nc.gpsimd.affine_select(
                            out=esb[:, par, 1, :, :], in_=esb[:, par, 1, :, :],
                            compare_op=mybir.AluOpType.is_ge, fill=0.0,
                            base=0, channel_multiplier=-1,
                            pattern=[[0, 2], [1, 128]],
                        )
                        if qi > 0:
                            nc.gpsimd.affine_select(
                                out=esb[:, par, 0, :, :],
                                in_=esb[:, par, 0, :, :],
                                compare_op=mybir.AluOpType.is_ge, fill=0.0,
                                base=-1, channel_multiplier=1,
                                pattern=[[0, 2], [-1, 128]],
                            )
                if qi == 0:
                    nc.gpsimd.memset(esb[:, :, 0, :, :], 0.0)
                if not split:
                    nc.gpsimd.affine_select(
                        out=esb[:, :, 1, :, :], in_=esb[:, :, 1, :, :],
                        compare_op=mybir.AluOpType.is_ge, fill=0.0,
                        base=0, channel_multiplier=-1,
                        pattern=[[0, 2], [0, 2], [1, 128]],
                    )
                    if qi > 0:
                        nc.gpsimd.affine_select(
                            out=esb[:, :, 0, :, :], in_=esb[:, :, 0, :, :],
                            compare_op=mybir.AluOpType.is_ge, fill=0.0,
                            base=-1, channel_multiplier=1,
                            pattern=[[0, 2], [0, 2], [-1, 128]],
                        )
                return esb

            def emit_O(qi, split=False):
                """[O'^T ; l] matmuls, then attT = O'^T / l.
                split: per-par O matmuls + normalize chain (tail latency)."""
                qsl = slice(qi * 128, (qi + 1) * 128)
                esb = esbs[qi]
                cis = [1] if qi == 0 else [0, 1]
                # col blocks ordered [h0, h2, h1, h3] so each parity's pair
                # of heads is contiguous for the normalize ops below
                psO = ps_o.tile([128, HPC, 128], F32, tag="O")
                l_sb = rlp.tile([64, HPC, 128], F32, tag="lsb")
                rl = rlp.tile([64, HPC, 128], F32, tag="rl")

                def o_mms(hs):
                    for h in hs:
                        mi, par = h // 2, h % 2
                        blk = par * 2 + mi
                        for i, ci in enumerate(cis):
                            kt = qi - 1 + ci
                            nc.tensor.matmul(
                                psO[:, blk, :],
                                lhsT=vA_sb[:, kt, h, :],
                                rhs=esb[:, par, ci, mi, :],
                                start=(i == 0), stop=(i == len(cis) - 1),
                            )

                def norm(s):
                    # stage l into SBUF first: the approx reciprocal's
                    # BITWISE_NOT seed needs the IEEE bit pattern, which a
                    # PSUM read does not reliably provide on hardware.
                    # l > 0 and well-scaled: far from the approx-fast edge
                    # cases; ~18 correct bits vs the exact recip (5x cost)
                    bs = slice(2 * s, 2 * s + 2)
                    nc.scalar.copy(out=l_sb[:, bs], in_=psO[64:128, bs, :])
                    nc.vector.reciprocal_approx_fast(
                        out=rl[:, bs], in_=l_sb[:, bs])
                    nc.vector.tensor_tensor(
                        attT_sb[s * 64:s * 64 + 64, :, qsl],
                        psO[0:64, bs, :],
                        rl[:, bs],
                        mybir.AluOpType.mult,
                    )

                if split:
                    # par 0 (heads 0,2 -> psO blks 0,1) finishes first;
                    # its normalize runs while par 1's O matmuls stream
                    o_mms([0, 2])
                    norm(0)
                    o_mms([1, 3])
                    norm(1)
                else:
                    o_mms(range(HPC))
                    nc.scalar.copy(out=l_sb, in_=psO[64:128, :, :])
                    nc.vector.reciprocal_approx_fast(out=rl, in_=l_sb)
                    for s in range(2):
                        nc.vector.tensor_tensor(
                            attT_sb[s * 64:s * 64 + 64, :, qsl],
                            psO[0:64, 2 * s:2 * s + 2, :],
                            rl[:, 2 * s:2 * s + 2, :],
                            mybir.AluOpType.mult,
                        )
                esbs.pop(qi)

            def st3_mm1(qi, nh, pool, tag="mm"):
                """one nh-half of stage3's matmuls (PE work only)."""
                tsl = slice(qi * 128, (qi + 1) * 128)
                po = pool.tile([128, 512], F32, tag=tag, name=f"po{nh}")
                for j in range(2):
                    nc.tensor.matmul(
                        po,
                        lhsT=attT_sb[:, j, tsl],
                        rhs=wout_sb[:, j, nh * 512:(nh + 1) * 512],
                        start=(j == 0), stop=(j == 1),
                    )
                return po

            def st3_mms(qi, pool=None):
                """stage3 matmuls only; casts/DMA split off so deferred
                tiles can fill tail PE gaps without their psum drains
                competing with the critical softmax chains."""
                pool = pool or ps_mm
                return [st3_mm1(qi, nh, pool) for nh in range(2)]

            def st3_out(qi, pos, act_only=False):
                """casts + output DMA for stage3.  act_only routes both
                casts to ACT (used for deferred tiles emitted after the
                last exp/l copies, when ACT is otherwise idle)."""
                tsl = slice(qi * 128, (qi + 1) * 128)
                o_sb = outw.tile([128, E], BF16, tag="osb")
                for nh, po in enumerate(pos):
                    base = nh * 512
                    if act_only:
                        nc.scalar.copy(
                            out=o_sb[:, base:base + 512], in_=po)
                        eng = nc.sync if nh == 0 else nc.scalar
                        eng.dma_start(
                            out=outp[tsl, base:base + 512],
                            in_=o_sb[:, base:base + 512])
                        continue
                    if qi >= NQT - 2:
                        # last tiles: split casts across DVE+ACT and DMA per
                        # quarter so the final drain is as short as possible;
                        # triggers split across Sync+ACT HWDGE queues so the
                        # 4 descriptors don't serialize on one engine
                        nc.vector.tensor_copy(
                            out=o_sb[:, base:base + 256], in_=po[:, 0:256])
                        nc.scalar.copy(
                            out=o_sb[:, base + 256:base + 512],
                            in_=po[:, 256:512])
                        for q, eng in ((0, nc.sync), (1, nc.scalar)):
                            eng.dma_start(
                                out=outp[tsl, base + q * 256:base + (q + 1) * 256],
                                in_=o_sb[:, base + q * 256:base + (q + 1) * 256])
                    else:
                        if nh == 0:
                            nc.vector.tensor_copy(
                                out=o_sb[:, 0:512], in_=po)
                        else:
                            nc.scalar.copy(out=o_sb[:, 512:1024], in_=po)
                        nc.sync.dma_start(
                            out=outp[tsl, base:base + 512],
                            in_=o_sb[:, base:base + 512])

            def stage3(qi):
                st3_out(qi, st3_mms(qi))

            # ---- software-pipelined main loop.  Attention runs two q-tiles
            # behind the qkv projections; O lags S by one q-tile (hides
            # exp+mask) and stage3 lags by two (hides the l-copy/
            # reciprocal/normalize chain). ----
            esbs = {}
            # stage3(3) is deferred into the tail: its matmuls fill the
            # PE wait on q15's exp chain (keeping the HAM clock gate at
            # 8/8 through the drain) and its casts run on ACT after the
            # last l-copies when ACT is idle
            DEFER = (3,)

            def pump(qi):
                split = qi >= NQT - 3
                esbs[qi] = emit_S(qi, split=split)
                osplit = qi - 1 >= NQT - 3
                if osplit and qi >= 2 and qi - 2 not in DEFER:
                    # tail pumps: stage3 first — its TT dep is long ripe,
                    # and its N=512 matmuls cover the wait on this pump's
                    # O mask chain instead of trailing it
                    stage3(qi - 2)
                    emit_O(qi - 1, split=True)
                else:
                    if qi >= 1:
                        emit_O(qi - 1, split=osplit)
                    if qi >= 2 and qi - 2 not in DEFER:
                        stage3(qi - 2)

            for ti in range(NT512):
                # k-chunks (mi 2,3) first so this block's own key tiles
                # have their S dependencies ready
                chunks = [lambda mi=mi: qk_chunk(ti, mi) for mi in (2, 3, 0, 1)]
                chunks += [lambda j=j: v_chunk(ti, j) for j in range(4)]
                if ti == 0:
                    # prologue pumps sit between the v-chunks so their
                    # exp/mask chains have matmul cover, like the steady state
                    qk_khalves0()
                    chunks[2]()
                    chunks[3]()
                    chunks[4]()
                    pump(0)
                    chunks[5]()
                    pump(1)
                    chunks[6]()
                    chunks[7]()
                elif ti < NT512 - 1:
                    for i in range(4):
                        chunks[2 * i]()
                        chunks[2 * i + 1]()
                        pump(4 * ti - 2 + i)
                else:
                    for i in range(3):
                        chunks[2 * i]()
                        chunks[2 * i + 1]()
                        pump(4 * ti - 2 + i)
                    pump(4 * ti + 1)
                    chunks[6]()
                    pump(4 * ti + 2)
                    chunks[7]()
                    pump(4 * ti + 3)
            # deferred stage3(3) split: nh0 covers O(15)'s mask wait,
            # nh1 the TT(15) wait between the final stage3s.  psum from
            # ps_o (its bufs WAR only prompt TT reads, never the late
            # filler casts); casts on ACT after the last l-copies; DMA
            # triggers spread over both HWDGE queues
            tslf = slice(DEFER[0] * 128, (DEFER[0] + 1) * 128)
            fo_sb = outw.tile([128, E], BF16, tag="osb", name="fo_sb")
            fill0 = st3_mm1(DEFER[0], 0, ps_o, tag="O")
            emit_O(NQT - 1, split=True)
            nc.scalar.copy(out=fo_sb[:, 0:512], in_=fill0)
            nc.sync.dma_start(out=outp[tslf, 0:512], in_=fo_sb[:, 0:512])
            stage3(NQT - 2)
            fill1 = st3_mm1(DEFER[0], 1, ps_o, tag="O")
            nc.scalar.copy(out=fo_sb[:, 512:1024], in_=fill1)
            nc.scalar.dma_start(out=outp[tslf, 512:1024], in_=fo_sb[:, 512:1024])
            stage3(NQT - 1)

    nc.finalize()
    return nc


_NC_CACHE = None


def _get_nc():
    global _NC_CACHE
    if _NC_CACHE is None:
        _NC_CACHE = build_bass()
    return _NC_CACHE


def _pack(a):
    """[E, m] -> [128, KO*m] in SBUF layout (p, ko, m), contiguous."""
    m = a.shape[1]
    return np.ascontiguousarray(
        a.reshape(KO, 128, m).transpose(1, 0, 2).reshape(128, KO * m))


def make_in_maps(x, w_qkv, w_out):
    x = np.asarray(x, dtype=np.float32)
    w_qkv = np.asarray(w_qkv, dtype=np.float32)
    w_out = np.asarray(w_out, dtype=np.float32)
    bf = ml_dtypes.bfloat16
    in_maps = []
    for c in range(N_CORES):
        b = c // 4
        hs = (c % 4) * HPC
        rows = slice(hs * D, (hs + HPC) * D)
        wq = w_qkv[0 * E:, :][rows] * SCALE    # fold 1/sqrt(D) into q
        wk = w_qkv[1 * E:, :][rows]
        wvs = w_qkv[2 * E:, :][rows]
        # [128, ko, t] prepack of x^T / weights (see build_bass DMA note)
        xp = x[b].T.astype(bf).reshape(KO, 128, T).transpose(1, 0, 2)
        wqkp = np.concatenate([wq, wk], axis=0).T.astype(bf).reshape(
            KO, 128, 512).transpose(1, 0, 2)
        in_maps.append({
            "x0a": np.ascontiguousarray(xp[:, :, 0:256]).reshape(128, -1),
            "x0b": np.ascontiguousarray(xp[:, :, 256:512]).reshape(128, -1),
            "x1": np.ascontiguousarray(xp[:, :, 512:1024]).reshape(128, -1),
            "x2": np.ascontiguousarray(xp[:, :, 1024:1536]).reshape(128, -1),
            "x3": np.ascontiguousarray(xp[:, :, 1536:2048]).reshape(128, -1),
            "wqkq": np.ascontiguousarray(wqkp[:, :, 0:256]).reshape(128, -1),
            "wqkk2": np.ascontiguousarray(wqkp[:, :, 256:384]).reshape(128, -1),
            "wqkk3": np.ascontiguousarray(wqkp[:, :, 384:512]).reshape(128, -1),
            "wvp": _pack(np.ascontiguousarray(wvs.T).astype(bf)),
            "wout": np.ascontiguousarray(w_out[:, rows].T).astype(bf),
        })
    return in_maps


def run(x, w_qkv, w_out, **spmd_kwargs):
    nc = _get_nc()
    in_maps = make_in_maps(x, w_qkv, w_out)
    res = run_bass_kernel_spmd(nc, in_maps, core_ids=list(range(N_CORES)),
                               **spmd_kwargs)
    outs = [r["outp"] for r in res.results]
    out = np.empty((B, T, E), dtype=np.float32)
    for b in range(B):
        acc = outs[4 * b].astype(np.float32)
        for c in range(4 * b + 1, 4 * b + 4):
            acc = acc + outs[c].astype(np.float32)
        out[b] = acc
    return out, res


def kernel(x, w_qkv, w_out):
    out, _ = run(x, w_qkv, w_out)
    return out



# revision 45
# speedup vs baseline: 1.0203x; 1.0203x over previous
"""Sliding-window (W=128) multi-head attention block for Trainium2, 8 cores.

Reference computation (B=2, T=2048, E=1024, H=16, D=64, W=128):
    qkv = x @ w_qkv.T ; split q,k,v ; heads ; att = softmax(mask(q k^T / 8)) v
    out = att_concat @ w_out.T

Sharding: data-parallel over B (2) x tensor-parallel over head groups (4),
so each of the 8 cores handles (one batch, 4 heads).  The output projection
is computed per-core against the 256 w_out columns belonging to its heads,
giving a partial [T, E] output (bf16); the host sums the 4 partials per
batch in f32.

Attention is computed in "transposed score" form to avoid PE transposes:
    S^T[k, q] = k^T.T @ q^T   (keys on partitions, d=64 contraction)
    E^T = exp(S^T)            (ACT, psum->sbuf bf16)
    mask -> 0 via affine_select on GpSimd (band structure is affine)
    [O'^T ; l] = [V | 1].T @ E^T  (l = softmax denominator, folded into the
                                   O matmul via ones-columns in the weights)
    attT = O'^T * (1/l)       (DVE reciprocal + multiply, psum in)
The two heads of a q^T/k^T pair chunk live at partitions 0:64 / 64:128 and
their K=64 S^T matmuls are row-tiled (tile_position auto-derived from the
base partition), so the pair runs concurrently on the PE array.

The 1/sqrt(D) scale is folded into the q weights on the host.
"""

import numpy as np
import ml_dtypes

import concourse.bass as bass
import concourse.bacc as bacc
import concourse.mybir as mybir
import concourse.tile as tile
from concourse.bass_utils import run_bass_kernel_spmd

B, T, E, H, W = 2, 2048, 1024, 16, 128
D = E // H            # 64
HPC = 4               # heads per core
N_CORES = 8
SCALE = 1.0 / float(np.sqrt(D))

BF16 = mybir.dt.bfloat16
F32 = mybir.dt.float32

KO = E // 128         # 8 contraction chunks
NQT = T // 128        # 16 query tiles
NT512 = T // 512      # 4 tiles for the projections

# Row-tiled K=64 S^T matmuls (pair-concurrent).  False falls back to
# zero-padded K=128 per-head kT (no partition-offset PE operands).
ROWPACK = True


def build_bass():
    nc = bacc.Bacc()
    # inputs prepacked on the host into SBUF layout ([p, ko, minor] row-
    # major) so every input DMA reads >=4KB contiguous runs per partition
    # (256-col rearranged slices of the natural layouts measured ~118GB/s
    # vs ~320GB/s for contiguous) and can be ordered by first use
    wqkk2 = nc.declare_dram_parameter("wqkk2", [128, KO * 128], BF16, isOutput=False)
    wqkk3 = nc.declare_dram_parameter("wqkk3", [128, KO * 128], BF16, isOutput=False)
    wqkq = nc.declare_dram_parameter("wqkq", [128, KO * 256], BF16, isOutput=False)
    wvp = nc.declare_dram_parameter("wvp", [128, KO * 256], BF16, isOutput=False)
    x0a = nc.declare_dram_parameter("x0a", [128, KO * 256], BF16, isOutput=False)
    x0b = nc.declare_dram_parameter("x0b", [128, KO * 256], BF16, isOutput=False)
    x1 = nc.declare_dram_parameter("x1", [128, KO * 512], BF16, isOutput=False)
    x2 = nc.declare_dram_parameter("x2", [128, KO * 512], BF16, isOutput=False)
    x3 = nc.declare_dram_parameter("x3", [128, KO * 512], BF16, isOutput=False)
    wout = nc.declare_dram_parameter("wout", [HPC * D, E], BF16, isOutput=False)
    outp = nc.declare_dram_parameter("outp", [T, E], BF16, isOutput=True)

    with tile.TileContext(nc) as tc:
        with (
            tc.tile_pool(name="persist", bufs=1) as persist,
            tc.tile_pool(name="work", bufs=3) as work,
            tc.tile_pool(name="rlp", bufs=2) as rlp,
            tc.tile_pool(name="outw", bufs=3) as outw,
            tc.tile_pool(name="ps_mm", bufs=2, space="PSUM") as ps_mm,
            tc.tile_pool(name="ps_s", bufs=2, space="PSUM") as ps_s,
            tc.tile_pool(name="ps_o", bufs=2, space="PSUM") as ps_o,
        ):
            # ---- persistent tiles ----
            wqk_sb = persist.tile([128, KO, 2 * HPC * D], BF16)
            wv_sb = persist.tile([128, KO, HPC * D], BF16)
            wout_sb = persist.tile([128, 2, E], BF16)
            xT_sb = persist.tile([128, KO, T], BF16)
            qkT_sb = persist.tile([128, 2, T], BF16)   # q^T pairs (scaled)
            if ROWPACK:
                kT_sb = persist.tile([128, 2, T], BF16)  # k^T pairs
            else:
                kT_sb = persist.tile([128, HPC, T], BF16)  # zero-padded k^T
                nc.vector.memset(kT_sb, 0.0)
            # V augmented with ones columns: [:, kt, h, 0:64]=V_h, 64:128=1
            vA_sb = persist.tile([128, NQT, HPC, 128], BF16)
            nc.gpsimd.memset(vA_sb[:, :, :, D:128], 1.0)
            attT_sb = persist.tile([128, 2, T], BF16)  # O^T, chunk j: heads 2j,2j+1

            # ---- PE warmup: dependency-free matmuls release the HAM clock
            # gate (K=4/8 -> 8/8 needs ~3.4us of sustained PE activity)
            # while the first input DMAs are still in flight ----
            wu_sb = persist.tile([128, 512], BF16)
            nc.vector.memset(wu_sb, 0.0)
            for wi in range(9):
                wu_ps = ps_o.tile([128, HPC, 128], F32, tag="O")
                nc.tensor.matmul(
                    wu_ps.rearrange("p a b -> p (a b)"), lhsT=wu_sb[:, 0:128],
                    rhs=wu_sb, start=True, stop=True)

            # ---- input DMAs, ordered by first use so the PE can stream
            # just behind the HBM reads (~320 GB/s): the ti=0 k-side
            # chunks (mi 2,3) need only wqk cols 256:512 + x tokens; the
            # token halves let the first matmuls start after ~1MB ----
            def ap3(t, m):
                return t[:, :].rearrange("p (ko m) -> p ko m", m=m)
            nc.sync.dma_start(out=wqk_sb[:, :, 256:384], in_=ap3(wqkk2, 128))
            nc.sync.dma_start(out=xT_sb[:, :, 0:256], in_=ap3(x0a, 256))
            nc.sync.dma_start(out=wqk_sb[:, :, 384:512], in_=ap3(wqkk3, 128))
            nc.sync.dma_start(out=xT_sb[:, :, 256:512], in_=ap3(x0b, 256))
            nc.sync.dma_start(out=wqk_sb[:, :, 0:256], in_=ap3(wqkq, 256))
            nc.sync.dma_start(out=wv_sb, in_=ap3(wvp, 256))
            nc.sync.dma_start(out=xT_sb[:, :, 512:1024], in_=ap3(x1, 512))
            nc.sync.dma_start(
                out=wout_sb, in_=wout[:, :].rearrange("(c p) m -> p c m", p=128))
            nc.sync.dma_start(out=xT_sb[:, :, 1024:1536], in_=ap3(x2, 512))
            nc.sync.dma_start(out=xT_sb[:, :, 1536:2048], in_=ap3(x3, 512))

            def qk_finish(ti, mi, ps):
                tsl = slice(ti * 512, (ti + 1) * 512)
                if mi < 2:
                    nc.vector.tensor_copy(out=qkT_sb[:, mi, tsl], in_=ps)
                elif ROWPACK:
                    nc.scalar.copy(out=kT_sb[:, mi - 2, tsl], in_=ps)
                else:
                    hp = (mi - 2) * 2
                    nc.scalar.copy(out=kT_sb[0:64, hp, tsl], in_=ps[0:64])
                    nc.scalar.copy(
                        out=kT_sb[64:128, hp + 1, tsl], in_=ps[64:128])

            def qk_chunk(ti, mi):
                ps = ps_mm.tile([128, 512], F32, tag="mm")
                for ko in range(KO):
                    nc.tensor.matmul(
                        ps,
                        lhsT=wqk_sb[:, ko, mi * 128:(mi + 1) * 128],
                        rhs=xT_sb[:, ko, ti * 512:(ti + 1) * 512],
                        start=(ko == 0), stop=(ko == KO - 1),
                    )
                qk_finish(ti, mi, ps)

            def qk_khalves0():
                # ti=0 k-chunks (mi 2,3) in N=256 token-halves, interleaved
                # so the PE streams just behind the input DMA: mi2 needs
                # only wqk cols 256:512 + x tokens 0:256 for its first MMs
                pss = {mi: ps_mm.tile([128, 512], F32, tag="mm",
                                      name=f"ps_kh{mi}")
                       for mi in (2, 3)}
                for lo, hi in ((0, 256), (256, 512)):
                    for mi in (2, 3):
                        for ko in range(KO):
                            nc.tensor.matmul(
                                pss[mi][:, lo:hi],
                                lhsT=wqk_sb[:, ko, mi * 128:(mi + 1) * 128],
                                rhs=xT_sb[:, ko, lo:hi],
                                start=(ko == 0), stop=(ko == KO - 1),
                            )
                for mi in (2, 3):
                    qk_finish(0, mi, pss[mi])

            def v_chunk(ti, j):
                t0 = ti * 512 + j * 128
                ps = ps_mm.tile([128, 512], F32, tag="mm")
                for ko in range(KO):
                    nc.tensor.matmul(
                        ps[:, 0:HPC * D],
                        lhsT=xT_sb[:, ko, t0:t0 + 128],
                        rhs=wv_sb[:, ko, :],
                        start=(ko == 0), stop=(ko == KO - 1),
                    )
                nc.vector.tensor_copy(
                    out=vA_sb[:, ti * 4 + j, :, 0:D],
                    in_=ps[:, 0:HPC * D])

            def emit_S(qi, split=False):
                """S^T matmuls + exp + band mask; returns E^T sbuf tile
                laid out [128 k, par, ci, mi, q] (head h = 2*mi + par).
                split: per-par exp/mask so par-0's O matmuls can start
                before par-1's softmax chain finishes (tail latency)."""
                qsl = slice(qi * 128, (qi + 1) * 128)
                cis = [1] if qi == 0 else [0, 1]
                psS = [ps_s.tile([128, 2, 2, 128], F32, tag=f"S{par}",
                                 name=f"psS{par}")
                       for par in range(2)]
                for ci in cis:
                    kt = qi - 1 + ci
                    ksl = slice(kt * 128, (kt + 1) * 128)
                    for mi in range(2):
                        for par in range(2):
                            prows = slice(par * 64, par * 64 + 64)
                            nc.tensor.matmul(
                                psS[par][:, ci, mi, :],
                                lhsT=kT_sb[prows, mi, ksl],
                                rhs=qkT_sb[prows, mi, qsl],
                                start=True, stop=True,
                            )
                esb = work.tile([128, 2, 2, 2, 128], BF16, tag="E")
                for par in range(2):
                    if qi == 0:
                        nc.scalar.activation(
                            out=esb[:, par, 1, :, :], in_=psS[par][:, 1, :, :],
                            func=mybir.ActivationFunctionType.Exp)
                    else:
                        nc.scalar.activation(
                            out=esb[:, par, :, :, :], in_=psS[par],
                            func=mybir.ActivationFunctionType.Exp)
                    if split:
                        nc.gpsimd.affine_select(
                            out=esb[:, par, 1, :, :], in_=esb[:, par, 1, :, :],
                            compare_op=mybir.AluOpType.is_ge, fill=0.0,
                            base=0, channel_multiplier=-1,
                            pattern=[[0, 2], [1, 128]],
                        )
                        if qi > 0:
                            nc.gpsimd.affine_select(
                                out=esb[:, par, 0, :, :],
                                in_=esb[:, par, 0, :, :],
                                compare_op=mybir.AluOpType.is_ge, fill=0.0,
                                base=-1, channel_multiplier=1,
                                pattern=[[0, 2], [-1, 128]],
                            )
                if qi == 0:
                    nc.gpsimd.memset(esb[:, :, 0, :, :], 0.0)
                if not split:
                    nc.gpsimd.affine_select(
                        out=esb[:, :, 1, :, :], in_=esb[:, :, 1, :, :],
                        compare_op=mybir.AluOpType.is_ge, fill=0.0,
                        base=0, channel_multiplier=-1,
                        pattern=[[0, 2], [0, 2], [1, 128]],
                    )
                    if qi > 0:
                        nc.gpsimd.affine_select(
                            out=esb[:, :, 0, :, :], in_=esb[:, :, 0, :, :],
                            compare_op=mybir.AluOpType.is_ge, fill=0.0,
                            base=-1, channel_multiplier=1,
                            pattern=[[0, 2], [0, 2], [-1, 128]],
                        )
                return esb

            def emit_O(qi, split=False):
                """[O'^T ; l] matmuls, then attT = O'^T / l.
                split: per-par O matmuls + normalize chain (tail latency)."""
                qsl = slice(qi * 128, (qi + 1) * 128)
                esb = esbs[qi]
                cis = [1] if qi == 0 else [0, 1]
                # col blocks ordered [h0, h2, h1, h3] so each parity's pair
                # of heads is contiguous for the normalize ops below
                psO = ps_o.tile([128, HPC, 128], F32, tag="O")
                l_sb = rlp.tile([64, HPC, 128], F32, tag="lsb")
                rl = rlp.tile([64, HPC, 128], F32, tag="rl")

                def o_mms(hs):
                    for h in hs:
                        mi, par = h // 2, h % 2
                        blk = par * 2 + mi
                        for i, ci in enumerate(cis):
                            kt = qi - 1 + ci
                            nc.tensor.matmul(
                                psO[:, blk, :],
                                lhsT=vA_sb[:, kt, h, :],
                                rhs=esb[:, par, ci, mi, :],
                                start=(i == 0), stop=(i == len(cis) - 1),
                            )

                def norm(s):
                    # stage l into SBUF first: the approx reciprocal's
                    # BITWISE_NOT seed needs the IEEE bit pattern, which a
                    # PSUM read does not reliably provide on hardware.
                    # l > 0 and well-scaled: far from the approx-fast edge
                    # cases; ~18 correct bits vs the exact recip (5x cost)
                    bs = slice(2 * s, 2 * s + 2)
                    nc.scalar.copy(out=l_sb[:, bs], in_=psO[64:128, bs, :])
                    nc.vector.reciprocal_approx_fast(
                        out=rl[:, bs], in_=l_sb[:, bs])
                    nc.vector.tensor_tensor(
                        attT_sb[s * 64:s * 64 + 64, :, qsl],
                        psO[0:64, bs, :],
                        rl[:, bs],
                        mybir.AluOpType.mult,
                    )

                if split:
                    # par 0 (heads 0,2 -> psO blks 0,1) finishes first;
                    # its normalize runs while par 1's O matmuls stream
                    o_mms([0, 2])
                    norm(0)
                    o_mms([1, 3])
                    norm(1)
                else:
                    o_mms(range(HPC))
                    nc.scalar.copy(out=l_sb, in_=psO[64:128, :, :])
                    nc.vector.reciprocal_approx_fast(out=rl, in_=l_sb)
                    for s in range(2):
                        nc.vector.tensor_tensor(
                            attT_sb[s * 64:s * 64 + 64, :, qsl],
                            psO[0:64, 2 * s:2 * s + 2, :],
                            rl[:, 2 * s:2 * s + 2, :],
                            mybir.AluOpType.mult,
                        )
                esbs.pop(qi)

            def st3_mm1(qi, nh, pool, tag="mm"):
                """one nh-half of stage3's matmuls (PE work only)."""
                tsl = slice(qi * 128, (qi + 1) * 128)
                po = pool.tile([128, 512], F32, tag=tag, name=f"po{nh}")
                for j in range(2):
                    nc.tensor.matmul(
                        po,
                        lhsT=attT_sb[:, j, tsl],
                        rhs=wout_sb[:, j, nh * 512:(nh + 1) * 512],
                        start=(j == 0), stop=(j == 1),
                    )
                return po

            def st3_mms(qi, pool=None):
                """stage3 matmuls only; casts/DMA split off so deferred
                tiles can fill tail PE gaps without their psum drains
                competing with the critical softmax chains."""
                pool = pool or ps_mm
                return [st3_mm1(qi, nh, pool) for nh in range(2)]

            def st3_out(qi, pos, act_only=False):
                """casts + output DMA for stage3.  act_only routes both
                casts to ACT (used for deferred tiles emitted after the
                last exp/l copies, when ACT is otherwise idle)."""
                tsl = slice(qi * 128, (qi + 1) * 128)
                o_sb = outw.tile([128, E], BF16, tag="osb")
                for nh, po in enumerate(pos):
                    base = nh * 512
                    if act_only:
                        nc.scalar.copy(
                            out=o_sb[:, base:base + 512], in_=po)
                        eng = nc.sync if nh == 0 else nc.scalar
                        eng.dma_start(
                            out=outp[tsl, base:base + 512],
                            in_=o_sb[:, base:base + 512])
                        continue
                    if qi >= NQT - 2:
                        # last tiles: split casts across DVE+ACT and DMA per
                        # quarter so the final drain is as short as possible;
                        # triggers split across Sync+ACT HWDGE queues so the
                        # 4 descriptors don't serialize on one engine
                        nc.vector.tensor_copy(
                            out=o_sb[:, base:base + 256], in_=po[:, 0:256])
                        nc.scalar.copy(
                            out=o_sb[:, base + 256:base + 512],
                            in_=po[:, 256:512])
                        for q, eng in ((0, nc.sync), (1, nc.scalar)):
                            eng.dma_start(
                                out=outp[tsl, base + q * 256:base + (q + 1) * 256],
                                in_=o_sb[:, base + q * 256:base + (q + 1) * 256])
                    else:
                        if nh == 0:
                            nc.vector.tensor_copy(
                                out=o_sb[:, 0:512], in_=po)
                        else:
                            nc.scalar.copy(out=o_sb[:, 512:1024], in_=po)
                        nc.sync.dma_start(
                            out=outp[tsl, base:base + 512],
                            in_=o_sb[:, base:base + 512])

            def stage3(qi):
                st3_out(qi, st3_mms(qi))

            # ---- software-pipelined main loop.  Attention runs two q-tiles
            # behind the qkv projections; O lags S by one q-tile (hides
            # exp+mask) and stage3 lags by two (hides the l-copy/
            # reciprocal/normalize chain). ----
            esbs = {}
            # stage3(3) is deferred into the tail: its matmuls fill the
            # PE wait on q15's exp chain (keeping the HAM clock gate at
            # 8/8 through the drain) and its casts run on ACT after the
            # last l-copies when ACT is idle
            DEFER = (3,)

            def pump(qi):
                split = qi >= NQT - 3
                esbs[qi] = emit_S(qi, split=split)
                osplit = qi - 1 >= NQT - 3
                if osplit and qi >= 2 and qi - 2 not in DEFER:
                    # tail pumps: stage3 first — its TT dep is long ripe,
                    # and its N=512 matmuls cover the wait on this pump's
                    # O mask chain instead of trailing it
                    stage3(qi - 2)
                    emit_O(qi - 1, split=True)
                else:
                    if qi >= 1:
                        emit_O(qi - 1, split=osplit)
                    if qi >= 2 and qi - 2 not in DEFER:
                        stage3(qi - 2)

            for ti in range(NT512):
                # k-chunks (mi 2,3) first so this block's own key tiles
                # have their S dependencies ready
                chunks = [lambda mi=mi: qk_chunk(ti, mi) for mi in (2, 3, 0, 1)]
                chunks += [lambda j=j: v_chunk(ti, j) for j in range(4)]
                if ti == 0:
                    # prologue pumps sit between the v-chunks so their
                    # exp/mask chains have matmul cover, like the steady state
                    qk_khalves0()
                    chunks[2]()
                    chunks[3]()
                    chunks[4]()
                    pump(0)
                    chunks[5]()
                    pump(1)
                    chunks[6]()
                    chunks[7]()
                elif ti < NT512 - 1:
                    for i in range(4):
                        chunks[2 * i]()
                        chunks[2 * i + 1]()
                        pump(4 * ti - 2 + i)
                else:
                    for i in range(3):
                        chunks[2 * i]()
                        chunks[2 * i + 1]()
                        pump(4 * ti - 2 + i)
                    pump(4 * ti + 1)
                    chunks[6]()
                    pump(4 * ti + 2)
                    chunks[7]()
                    pump(4 * ti + 3)
            fill_a = st3_mms(DEFER[0])
            emit_O(NQT - 1, split=True)
            st3_out(DEFER[0], fill_a, act_only=True)
            stage3(NQT - 2)
            stage3(NQT - 1)

    nc.finalize()
    return nc


_NC_CACHE = None


def _get_nc():
    global _NC_CACHE
    if _NC_CACHE is None:
        _NC_CACHE = build_bass()
    return _NC_CACHE


def _pack(a):
    """[E, m] -> [128, KO*m] in SBUF layout (p, ko, m), contiguous."""
    m = a.shape[1]
    return np.ascontiguousarray(
        a.reshape(KO, 128, m).transpose(1, 0, 2).reshape(128, KO * m))


def make_in_maps(x, w_qkv, w_out):
    x = np.asarray(x, dtype=np.float32)
    w_qkv = np.asarray(w_qkv, dtype=np.float32)
    w_out = np.asarray(w_out, dtype=np.float32)
    bf = ml_dtypes.bfloat16
    in_maps = []
    for c in range(N_CORES):
        b = c // 4
        hs = (c % 4) * HPC
        rows = slice(hs * D, (hs + HPC) * D)
        wq = w_qkv[0 * E:, :][rows] * SCALE    # fold 1/sqrt(D) into q
        wk = w_qkv[1 * E:, :][rows]
        wvs = w_qkv[2 * E:, :][rows]
        # [128, ko, t] prepack of x^T / weights (see build_bass DMA note)
        xp = x[b].T.astype(bf).reshape(KO, 128, T).transpose(1, 0, 2)
        wqkp = np.concatenate([wq, wk], axis=0).T.astype(bf).reshape(
            KO, 128, 512).transpose(1, 0, 2)
        in_maps.append({
            "x0a": np.ascontiguousarray(xp[:, :, 0:256]).reshape(128, -1),
            "x0b": np.ascontiguousarray(xp[:, :, 256:512]).reshape(128, -1),
            "x1": np.ascontiguousarray(xp[:, :, 512:1024]).reshape(128, -1),
            "x2": np.ascontiguousarray(xp[:, :, 1024:1536]).reshape(128, -1),
            "x3": np.ascontiguousarray(xp[:, :, 1536:2048]).reshape(128, -1),
            "wqkq": np.ascontiguousarray(wqkp[:, :, 0:256]).reshape(128, -1),
            "wqkk2": np.ascontiguousarray(wqkp[:, :, 256:384]).reshape(128, -1),
            "wqkk3": np.ascontiguousarray(wqkp[:, :, 384:512]).reshape(128, -1),
            "wvp": _pack(np.ascontiguousarray(wvs.T).astype(bf)),
            "wout": np.ascontiguousarray(w_out[:, rows].T).astype(bf),
        })
    return in_maps


def run(x, w_qkv, w_out, **spmd_kwargs):
    nc = _get_nc()
    in_maps = make_in_maps(x, w_qkv, w_out)
    res = run_bass_kernel_spmd(nc, in_maps, core_ids=list(range(N_CORES)),
                               **spmd_kwargs)
    outs = [r["outp"] for r in res.results]
    out = np.empty((B, T, E), dtype=np.float32)
    for b in range(B):
        acc = outs[4 * b].astype(np.float32)
        for c in range(4 * b + 1, 4 * b + 4):
            acc = acc + outs[c].astype(np.float32)
        out[b] = acc
    return out, res


def kernel(x, w_qkv, w_out):
    out, _ = run(x, w_qkv, w_out)
    return out

